# revision 33
# baseline (speedup 1.0000x reference)
"""Trainium2 Bass kernel for masked attention softmax (ragged sequences).

Reference computation (per batch b):
    qp[k]   = sum_q query[b,0,q] * w[k,q]
    att[s]  = sum_k qp[k] * keys[b,s,k]
    score   = where(s < seq_len[b], att, NEG_INF)
    out[b]  = softmax(score)            # over s axis

Strategy (v3 -- DVE tree-fold for long slots + PE per-batch matvec for
short slots):
  - Data-parallel over batch across 8 cores (512 batches/core, 4 slot
    tiles of 128 on the partition dim).  Batches sorted by seq_len
    descending and dealt round-robin so slot j has the same extent E_j on
    every core; keys zero-padded to E_j.  All big tensors fp16.
  - Slots 0..1 (long, E>128) on the DVE: scalar_tensor_tensor (the v0
    kernel's op) has NO DVE perf mode, but tensor_tensor runs 2x_1P with
    all-fp16 operands, so per 64-position chunk:
      prod = kt * qp_bcast        (TT mult fp16 2x, 0-stride qp broadcast)
      fold 128 -> 64 -> 32 -> 16 -> 8   (TT add fp16 2x)
      att  = tensor_reduce(axis=X, fp32)  (fp32 accumulator finishes)
  - Slots 2..3 (E<=128) on the otherwise-idle PE: per batch one matvec
    (stationary = that batch's qpT column, moving = its keysT block) into
    PSUM partition 32r via column tile_position, 16 batches per PSUM
    bank.  ACT evacuates the 4 used rows, then an SBUF->SBUF HWDGE DMA
    re-gathers them into batch-on-partitions layout [128,E] (the one
    layout every later stage needs; DMA is the only unit that can cross
    partitions).  ~60ns/batch on PE replaces ~140ns/position on DVE.
  - Mask: att += maskpen (0 valid / -1e9 padded) before exp, so padded
    positions contribute exp(-1e9)=0 to the softmax sum exactly (a
    pad-count subtraction cancels catastrophically for tiny exp-sums).
  - softmax per slot: ACT exp with accum_out, DVE reciprocal, ACT scale,
    fp16 output.  No max-subtraction: |att| <= ~60, exp finite in fp32,
    softmax shift-invariant.  seq_len==0 rows fixed on host.
  - qp on device (PE): per slot one fp16 matmul [qT | wT] -> PSUM fp32,
    ACT cast to fp16 ([b,k] for DVE slots, [k,b] for PE slots).

  Measured on trn2 (8 cores): 77.4-78.4us HW exec typical (one 83us
  contention outlier observed), max rel err 1.02e-2
  (gate 2e-2; the error is dominated by the fp16 tree-fold rounding on
  the DVE slots -- the PE-slot dot products accumulate in fp32).
  Baseline STT kernel: 129.7us.  The first-slot DMA ramp grows ~1.5x per
  chunk (8,12,16,24,32,48) -- matching HBM delivery (~91ns/pos) against
  DVE consumption (~138ns/pos); a 2x ramp starves the DVE ~4us.
  Rejected (measured): fold-to-4 extra level (instr+sem overhead eats the
  384-cycle saving), deeper keys prefetch bufs=5 (SBUF contention slows
  ALL concurrent DVE ops ~1.2x), PE path for the E~150 slot via 2-part
  split (true per-matvec cost ~250ns MM + ~100ns LDWEIGHTS, no LDW
  pull-ahead with full-row stationaries -> 772 matvecs PE-bound, 102us),
  PEB=16 finer PE chunks (doubles DMA-issue instrs on the ACT ring,
  head-of-line sems 12->30us, 94us), CH=96 chunks with bufs=3 (88us),
  ACT-side reciprocal via exp(-ln(x)) (inf outputs, and slower).
  The 1:1 DVE:PE chunk interleave after the ramp + deferring the mp load
  past the ramp pulls the PE slots' softmax tails into the mid-stream
  DVE queue instead of serializing them at the end (-2.5us vs 2:1).
"""

import sys

import numpy as np

sys.path.insert(0, "/opt/trn_rl_repo")

import concourse.bass as bass
import concourse.tile as tile
from concourse import bacc, mybir
from concourse.bass_utils import run_bass_kernel_spmd


def _install_trace_shims():
    """The agent image lacks ``antenv.axon_hooks``, so trace=True silently
    degrades.  Recreate the module and register the ctypes NTFF hook from
    trn_agent_boot; also make artifact upload failure non-fatal."""
    try:
        import types

        import antenv
        from concourse import bass_utils as _bu

        if "antenv.axon_hooks" not in sys.modules:
            mod = types.ModuleType("antenv.axon_hooks")
            mod._hook = None
            mod.set_axon_ntff_profile_hook = lambda h: setattr(mod, "_hook", h)
            mod.get_axon_ntff_profile_hook = lambda: mod._hook
            sys.modules["antenv.axon_hooks"] = mod
            antenv.axon_hooks = mod
            from trn_agent_boot.trn_boot import _ntff_profile_via_ctypes

            mod.set_axon_ntff_profile_hook(
                _ntff_profile_via_ctypes("/opt/axon/libaxon_pjrt.so")
            )

        _orig_upload = _bu.upload_artifacts

        def _safe_upload(tmpdir):
            try:
                return _orig_upload(tmpdir)
            except Exception:
                return "local://" + str(tmpdir)

        _bu.upload_artifacts = _safe_upload
    except Exception:
        pass


_install_trace_shims()

B, S, KD, QD = 4096, 200, 128, 128
NCORES = 8
P = 128
PB = B // NCORES           # batches per core
NTILES = PB // P           # slot tiles per core
CH = 64                    # s-positions per keys DMA / DVE chunk
PEB = 32                   # batches per PE-slot DMA chunk (= 2 PSUM banks)
MASK_NEG = -1.0e9

LAST_RESULTS = None
_nc_cache = {}


def _pe_slots(s_exts):
    return [j for j in range(NTILES) if s_exts[j] <= P]


def _chunks_for(E, first_slot):
    plan = []
    c0 = 0
    if first_slot:
        for ch in (16, 16, 24, 32, 48):
            if c0 + ch <= E:
                plan.append((c0, ch))
                c0 += ch
    while c0 < E:
        ch = min(CH, E - c0)
        plan.append((c0, ch))
        c0 += ch
    return plan


def _build(s_exts):
    f16 = mybir.dt.float16
    f32 = mybir.dt.float32
    SE = sum(s_exts)
    offs = np.cumsum([0] + list(s_exts[:-1])).tolist()
    pe_slots = _pe_slots(s_exts)
    dve_slots = [j for j in range(NTILES) if j not in pe_slots]
    SE_DVE = sum(s_exts[j] for j in dve_slots)
    PE_COLS = sum(P * s_exts[j] for j in pe_slots)
    pe_off = {}
    acc = 0
    for j in pe_slots:
        pe_off[j] = acc
        acc += P * s_exts[j]
    dve_off = {}
    acc = 0
    for j in dve_slots:
        dve_off[j] = acc
        acc += s_exts[j]

    nc = bacc.Bacc("TRN2", target_bir_lowering=False, debug=False)

    keys_d = nc.dram_tensor("keys", [P, max(SE_DVE, 1), KD], f16, kind="ExternalInput")
    ktp_d = nc.dram_tensor("ktp", [KD, max(PE_COLS, 1)], f16, kind="ExternalInput")
    qw_d = nc.dram_tensor("qw", [QD, NTILES, P + KD], f16, kind="ExternalInput")
    qt4_d = nc.dram_tensor(
        "qt4", [QD, max(len(pe_slots), 1), 4 * P], f16, kind="ExternalInput"
    )
    mp_d = nc.dram_tensor("mp", [P, SE], f32, kind="ExternalInput")
    out_d = nc.dram_tensor("out", [P, SE], f16, kind="ExternalOutput")

    with nc.allow_low_precision(reason="fp16 tree-fold; tensor_reduce tail is fp32"):
        with tile.TileContext(nc) as tc:
            with (
                tc.tile_pool(name="keys", bufs=4) as keysp,
                tc.tile_pool(name="ktpool", bufs=3) as ktpp,
                tc.tile_pool(name="prod", bufs=2) as prodp,
                tc.tile_pool(name="h16", bufs=2) as h16p,
                tc.tile_pool(name="h32", bufs=2) as h32p,
                tc.tile_pool(name="small", bufs=2) as smallp,
                tc.tile_pool(name="soft", bufs=3) as softp,
                tc.tile_pool(name="evac", bufs=3) as evacp,
                tc.tile_pool(name="qpp", bufs=NTILES) as qpp,
                tc.tile_pool(name="psum", bufs=2, space=bass.MemorySpace.PSUM) as psump,
                tc.tile_pool(name="psbank", bufs=4, space=bass.MemorySpace.PSUM) as psbankp,
            ):
                qw = smallp.tile([QD, NTILES, P + KD], f16, tag="qw")
                nc.sync.dma_start(qw[:], qw_d[:])
                mp_t = smallp.tile([P, SE], f32, tag="mp")

                # qp for ALL slots up-front (PE otherwise idle):
                # [b,k] for DVE slots, transposed [k,b] for PE slots.
                qt4 = smallp.tile([QD, max(len(pe_slots), 1), 4 * P], f16, tag="qt4")
                if pe_slots:
                    nc.scalar.dma_start(qt4[:], qt4_d[:])
                qps = {}
                for j in range(NTILES):
                    if j in pe_slots:
                        jj = pe_slots.index(j)
                        qp_ps4 = psump.tile([P, 4 * P], f32, tag="qp_ps")
                        nc.tensor.matmul(
                            qp_ps4[:], qw[:, j, P : P + KD], qt4[:, jj, :],
                            start=True, stop=True,
                        )
                        qp = qpp.tile(
                            [P, 4 * P], f16, name=f"qp{j}", tag=f"qp{j}"
                        )
                        nc.scalar.copy(qp[:], qp_ps4[:])
                    else:
                        qp_ps = psump.tile([P, KD], f32, tag="qp_ps")
                        nc.tensor.matmul(
                            qp_ps[:], qw[:, j, :P], qw[:, j, P : P + KD],
                            start=True, stop=True,
                        )
                        qp = qpp.tile([P, KD], f16, name=f"qp{j}", tag=f"qp{j}")
                        nc.scalar.copy(qp[:], qp_ps[:])
                    qps[j] = qp

                # merged chunk plan: DVE keys chunks + PE batch-block
                # chunks, interleaved 2:1 so the DVE never starves while
                # the PE stream still lands early enough to overlap.
                dve_plan = []
                for j in dve_slots:
                    for c0, ch in _chunks_for(s_exts[j], j == dve_slots[0]):
                        dve_plan.append(("dve", j, c0, ch))
                pe_plan = []
                for j in pe_slots:
                    for b0 in range(0, P, PEB):
                        pe_plan.append(("pe", j, b0, PEB))
                plan = []
                di, pi = 0, 0
                while di < len(dve_plan) or pi < len(pe_plan):
                    n_dve = 2 if di < 5 else 1
                    for _ in range(n_dve):
                        if di < len(dve_plan):
                            plan.append(dve_plan[di]); di += 1
                    if pi < len(pe_plan) and di >= 5:
                        plan.append(pe_plan[pi]); pi += 1
                    if di >= len(dve_plan) and pi < len(pe_plan):
                        plan.append(pe_plan[pi]); pi += 1

                atts = {}
                done_pos = {j: 0 for j in range(NTILES)}
                qidx = 0

                # softmax tail in two phases: phase 1 (mask + exp) at
                # slot completion, phase 2 (recip + scale + output) two
                # plan items later -- the in-order DVE queue would
                # otherwise head-block on reciprocal waiting for ACT's exp
                soft_state = {}
                pending2 = []

                mp_state = {"loaded": False}

                def load_mp():
                    if not mp_state["loaded"]:
                        nc.scalar.dma_start(mp_t[:], mp_d[:])
                        mp_state["loaded"] = True

                def softmax_phase1(j):
                    load_mp()
                    E = s_exts[j]
                    off = offs[j]
                    att = atts[j]
                    atm = softp.tile([P, E], f32, name=f"atm{j}", tag="atm")
                    nc.vector.tensor_tensor(
                        atm[:], att[:], mp_t[:, off : off + E],
                        op=mybir.AluOpType.add,
                    )
                    e_t = softp.tile([P, E], f32, name=f"e{j}", tag="e")
                    ssum = softp.tile([P, 1], f32, name=f"ssum{j}", tag="ssum")
                    nc.scalar.activation(
                        e_t[:], atm[:], mybir.ActivationFunctionType.Exp,
                        bias=0.0, scale=1.0, accum_out=ssum[:],
                    )
                    soft_state[j] = (e_t, ssum)

                def softmax_phase2(j):
                    E = s_exts[j]
                    off = offs[j]
                    e_t, ssum = soft_state[j]
                    rec = softp.tile([P, 1], f32, name=f"rec{j}", tag="rec")
                    nc.vector.reciprocal(rec[:], ssum[:])
                    o_t = softp.tile([P, E], f16, name=f"o{j}", tag="o")
                    nc.scalar.mul(o_t[:], e_t[:], rec[:])
                    out_eng = nc.sync if j == NTILES - 1 else nc.gpsimd
                    out_eng.dma_start(out_d[:, off : off + E], o_t[:])

                def softmax_tail(j, i):
                    softmax_phase1(j)
                    pending2.append((i + 2, j))

                for i, item in enumerate(plan):
                    while pending2 and pending2[0][0] <= i:
                        softmax_phase2(pending2.pop(0)[1])
                    kind, j, a0, an = item
                    E = s_exts[j]
                    if j not in atts:
                        atts[j] = softp.tile(
                            [P, E], f32, name=f"att{j}", tag=f"att{j}"
                        )
                    att = atts[j]
                    if i == 4:
                        load_mp()
                    dma_eng = nc.scalar if (qidx % 2 == 0) else nc.sync
                    qidx += 1

                    if kind == "dve":
                        c0, ch = a0, an
                        off = dve_off[j]
                        qp = qps[j]
                        kt = keysp.tile([P, CH, KD], f16, tag="kt")
                        dma_eng.dma_start(
                            kt[:, :ch, :], keys_d[:, off + c0 : off + c0 + ch, :]
                        )
                        prod = prodp.tile([P, CH, KD], f16, tag="prod")
                        nc.vector.tensor_tensor(
                            prod[:, :ch, :],
                            kt[:, :ch, :],
                            qp[:].unsqueeze(1).broadcast_to([P, ch, KD]),
                            op=mybir.AluOpType.mult,
                        )
                        h1 = h16p.tile([P, CH, 64], f16, tag="h1")
                        nc.vector.tensor_tensor(
                            h1[:, :ch, :], prod[:, :ch, 0:64], prod[:, :ch, 64:128],
                            op=mybir.AluOpType.add,
                        )
                        h2 = h32p.tile([P, CH, 32], f16, tag="h2")
                        nc.vector.tensor_tensor(
                            h2[:, :ch, :], h1[:, :ch, 0:32], h1[:, :ch, 32:64],
                            op=mybir.AluOpType.add,
                        )
                        h3 = h32p.tile([P, CH, 16], f16, tag="h3")
                        nc.vector.tensor_tensor(
                            h3[:, :ch, :], h2[:, :ch, 0:16], h2[:, :ch, 16:32],
                            op=mybir.AluOpType.add,
                        )
                        h4 = h16p.tile([P, CH, 8], f16, tag="h4")
                        nc.vector.tensor_tensor(
                            h4[:, :ch, :], h3[:, :ch, 0:8], h3[:, :ch, 8:16],
                            op=mybir.AluOpType.add,
                        )
                        nc.vector.tensor_reduce(
                            att[:, c0 : c0 + ch], h4[:, :ch, :],
                            axis=mybir.AxisListType.X, op=mybir.AluOpType.add,
                        )
                        done_pos[j] += ch
                        if done_pos[j] == E:
                            softmax_tail(j, i)
                    else:
                        b0 = a0
                        qpT = qps[j]
                        ktp = ktpp.tile([KD, PEB * E], f16, tag="ktp")
                        dma_eng.dma_start(
                            ktp[:],
                            ktp_d[:, pe_off[j] + b0 * E : pe_off[j] + (b0 + PEB) * E],
                        )
                        # E>128 slots split each batch into parts (0,128) +
                        # (128,E); each 16-batch group fills one PSUM bank
                        # per part: batch b -> strip r=(b%16)//4 (psum
                        # partitions 32r..32r+4, 4x-replicated stationary),
                        # block i=b%4 (free cols [128i, 128i+pw)).
                        parts = [(0, min(E, P))]
                        if E > P:
                            parts.append((P, E - P))
                        for kk in range(PEB // 16):
                            for p0, pw in parts:
                                bank = psbankp.tile([P, 4, P], f32, tag="bank")
                                for bl in range(16):
                                    b = b0 + kk * 16 + bl
                                    r, ii = bl // 4, bl % 4
                                    u = kk * 16 + bl
                                    nc.tensor.matmul(
                                        bank[32 * r : 32 * r + 4, ii, 0:pw],
                                        qpT[:, 4 * b : 4 * b + 4],
                                        ktp[:, u * E + p0 : u * E + p0 + pw],
                                        start=True, stop=True,
                                        tile_position=(0, 32 * r),
                                    )
                                # full-partition evac: compute engines cannot
                                # stride the partition dim (only rows 32r
                                # carry data; the rest is ignored)
                                ev = evacp.tile([P, 4, P], f32, tag="ev")
                                nc.scalar.copy(
                                    ev[:, :, 0:pw], bank[:, :, 0:pw]
                                )
                                # partition-crossing re-gather: dest
                                # partition 16k+4r+i <- (strip 32r, block i);
                                # DMA is the only unit that crosses partitions
                                nc.sync.dma_start(
                                    att[b0 + kk * 16 : b0 + kk * 16 + 16, p0 : p0 + pw],
                                    ev[0:97:32, 0:4, 0:pw],
                                )
                        done_pos[j] += PEB
                        if done_pos[j] == P:
                            softmax_tail(j, i)
                while pending2:
                    softmax_phase2(pending2.pop(0)[1])
    nc.compile()
    return nc


def _prep(query, keys, seq_len, w):
    query = np.asarray(query)
    keys = np.asarray(keys)
    w = np.asarray(w)
    lens = np.asarray(seq_len).reshape(B).astype(np.int64)

    order = np.argsort(-lens, kind="stable")
    gp = NCORES * P  # batches per slot across all cores
    slot_max = [int(lens[order[j * gp : (j + 1) * gp]].max()) for j in range(NTILES)]
    s_exts = tuple(min(S, max(1, m)) for m in slot_max)
    SE = sum(s_exts)
    pe_slots = _pe_slots(s_exts)
    dve_slots = [j for j in range(NTILES) if j not in pe_slots]
    SE_DVE = sum(s_exts[j] for j in dve_slots)
    PE_COLS = sum(P * s_exts[j] for j in pe_slots)

    perms = []
    for c in range(NCORES):
        perms.append(
            np.concatenate(
                [order[j * gp : (j + 1) * gp][c::NCORES] for j in range(NTILES)]
            )
        )

    keys16 = keys.astype(np.float16)
    q16 = query[:, 0, :].astype(np.float16)
    wT16 = np.ascontiguousarray(w.T).astype(np.float16)
    arange_s = np.arange(S)[None, :]

    in_maps = []
    for c in range(NCORES):
        pc = perms[c]
        ka = np.zeros((P, max(SE_DVE, 1), KD), dtype=np.float16)
        ktp = np.zeros((KD, max(PE_COLS, 1)), dtype=np.float16)
        qt4 = np.zeros((QD, max(len(pe_slots), 1), 4 * P), dtype=np.float16)
        mp = np.zeros((P, SE), dtype=np.float32)
        qw = np.empty((QD, NTILES, P + KD), dtype=np.float16)
        off_all = 0
        off_dve = 0
        off_pe = 0
        for j in range(NTILES):
            E = s_exts[j]
            rows = pc[j * P : (j + 1) * P]
            sl = np.minimum(lens[rows], E)
            blk = keys16[rows, :E, :]
            blk = np.where((arange_s[:, :E, None] < sl[:, None, None]), blk, 0)
            if j in pe_slots:
                # [k, b, s] batch-major column blocks
                ktp[:, off_pe : off_pe + P * E] = blk.transpose(2, 0, 1).reshape(
                    KD, P * E
                )
                off_pe += P * E
                qt4[:, pe_slots.index(j), :] = np.repeat(q16[rows].T, 4, axis=1)
            else:
                ka[:, off_dve : off_dve + E, :] = blk
                off_dve += E
            mp[:, off_all : off_all + E] = np.where(
                arange_s[:, :E] < sl[:, None], 0.0, np.float32(MASK_NEG)
            )
            qw[:, j, :P] = q16[rows].T
            qw[:, j, P:] = wT16
            off_all += E
        in_maps.append({"keys": ka, "ktp": ktp, "qw": qw, "qt4": qt4, "mp": mp})
    return lens, s_exts, perms, in_maps


def kernel(query, keys, seq_len, w):
    global LAST_RESULTS
    lens, s_exts, perms, in_maps = _prep(query, keys, seq_len, w)

    nc = _nc_cache.get(s_exts)
    if nc is None:
        nc = _build(s_exts)
        _nc_cache[s_exts] = nc

    res = run_bass_kernel_spmd(nc, in_maps, core_ids=list(range(NCORES)))
    LAST_RESULTS = res

    out = np.zeros((B, S), dtype=np.float32)
    for c in range(NCORES):
        dev = np.asarray(res.results[c]["out"]).astype(np.float32)
        pc = perms[c]
        off = 0
        for j in range(NTILES):
            E = s_exts[j]
            rows = pc[j * P : (j + 1) * P]
            out[rows, :E] = dev[:, off : off + E]
            off += E
    # zero masked/padded positions, then fix seq_len==0 rows (uniform).
    out = np.where(np.arange(S)[None, :] < lens[:, None], out, 0.0)
    out[lens == 0, :] = np.float32(1.0 / S)
    return out


# revision 34
# speedup vs baseline: 1.0255x; 1.0255x over previous
"""Trainium2 Bass kernel for masked attention softmax (ragged sequences).

Reference computation (per batch b):
    qp[k]   = sum_q query[b,0,q] * w[k,q]
    att[s]  = sum_k qp[k] * keys[b,s,k]
    score   = where(s < seq_len[b], att, NEG_INF)
    out[b]  = softmax(score)            # over s axis

Strategy (v3 -- DVE tree-fold for long slots + PE per-batch matvec for
short slots):
  - Data-parallel over batch across 8 cores (512 batches/core, 4 slot
    tiles of 128 on the partition dim).  Batches sorted by seq_len
    descending and dealt round-robin so slot j has the same extent E_j on
    every core; keys zero-padded to E_j.  All big tensors fp16.
  - Slots 0..1 (long, E>128) on the DVE: scalar_tensor_tensor (the v0
    kernel's op) has NO DVE perf mode, but tensor_tensor runs 2x_1P with
    all-fp16 operands, so per 64-position chunk:
      prod = kt * qp_bcast        (TT mult fp16 2x, 0-stride qp broadcast)
      fold 128 -> 64 -> 32 -> 16 -> 8   (TT add fp16 2x)
      att  = tensor_reduce(axis=X, fp32)  (fp32 accumulator finishes)
  - Slots 2..3 (E<=128) on the otherwise-idle PE: per batch one matvec
    (stationary = that batch's qpT column, moving = its keysT block) into
    PSUM partition 32r via column tile_position, 16 batches per PSUM
    bank.  ACT evacuates the 4 used rows, then an SBUF->SBUF HWDGE DMA
    re-gathers them into batch-on-partitions layout [128,E] (the one
    layout every later stage needs; DMA is the only unit that can cross
    partitions).  ~60ns/batch on PE replaces ~140ns/position on DVE.
  - Mask: att += maskpen (0 valid / -1e9 padded) before exp, so padded
    positions contribute exp(-1e9)=0 to the softmax sum exactly (a
    pad-count subtraction cancels catastrophically for tiny exp-sums).
  - softmax per slot: ACT exp with accum_out, DVE reciprocal, ACT scale,
    fp16 output.  No max-subtraction: |att| <= ~60, exp finite in fp32,
    softmax shift-invariant.  seq_len==0 rows fixed on host.
  - qp on device (PE): per slot one fp16 matmul [qT | wT] -> PSUM fp32,
    ACT cast to fp16 ([b,k] for DVE slots, [k,b] for PE slots).

  Measured on trn2 (8 cores): 77.4-78.4us HW exec typical (one 83us
  contention outlier observed), max rel err 1.02e-2
  (gate 2e-2; the error is dominated by the fp16 tree-fold rounding on
  the DVE slots -- the PE-slot dot products accumulate in fp32).
  Baseline STT kernel: 129.7us.  The first-slot DMA ramp grows ~1.5x per
  chunk (8,12,16,24,32,48) -- matching HBM delivery (~91ns/pos) against
  DVE consumption (~138ns/pos); a 2x ramp starves the DVE ~4us.
  Rejected (measured): fold-to-4 extra level (instr+sem overhead eats the
  384-cycle saving), deeper keys prefetch bufs=5 (SBUF contention slows
  ALL concurrent DVE ops ~1.2x), PE path for the E~150 slot via 2-part
  split (true per-matvec cost ~250ns MM + ~100ns LDWEIGHTS, no LDW
  pull-ahead with full-row stationaries -> 772 matvecs PE-bound, 102us),
  PEB=16 finer PE chunks (doubles DMA-issue instrs on the ACT ring,
  head-of-line sems 12->30us, 94us), CH=96 chunks with bufs=3 (88us),
  ACT-side reciprocal via exp(-ln(x)) (inf outputs, and slower).
  The 1:1 DVE:PE chunk interleave after the ramp + deferring the mp load
  past the ramp pulls the PE slots' softmax tails into the mid-stream
  DVE queue instead of serializing them at the end (-2.5us vs 2:1).
"""

import sys

import numpy as np

sys.path.insert(0, "/opt/trn_rl_repo")

import concourse.bass as bass
import concourse.tile as tile
from concourse import bacc, mybir
from concourse.bass_utils import run_bass_kernel_spmd


def _install_trace_shims():
    """The agent image lacks ``antenv.axon_hooks``, so trace=True silently
    degrades.  Recreate the module and register the ctypes NTFF hook from
    trn_agent_boot; also make artifact upload failure non-fatal."""
    try:
        import types

        import antenv
        from concourse import bass_utils as _bu

        if "antenv.axon_hooks" not in sys.modules:
            mod = types.ModuleType("antenv.axon_hooks")
            mod._hook = None
            mod.set_axon_ntff_profile_hook = lambda h: setattr(mod, "_hook", h)
            mod.get_axon_ntff_profile_hook = lambda: mod._hook
            sys.modules["antenv.axon_hooks"] = mod
            antenv.axon_hooks = mod
            from trn_agent_boot.trn_boot import _ntff_profile_via_ctypes

            mod.set_axon_ntff_profile_hook(
                _ntff_profile_via_ctypes("/opt/axon/libaxon_pjrt.so")
            )

        _orig_upload = _bu.upload_artifacts

        def _safe_upload(tmpdir):
            try:
                return _orig_upload(tmpdir)
            except Exception:
                return "local://" + str(tmpdir)

        _bu.upload_artifacts = _safe_upload
    except Exception:
        pass


_install_trace_shims()

B, S, KD, QD = 4096, 200, 128, 128
NCORES = 8
P = 128
PB = B // NCORES           # batches per core
NTILES = PB // P           # slot tiles per core
CH = 64                    # s-positions per keys DMA / DVE chunk
PEB = 32                   # batches per PE-slot DMA chunk (= 2 PSUM banks)
MASK_NEG = -1.0e9

LAST_RESULTS = None
_nc_cache = {}


def _pe_slots(s_exts):
    return [j for j in range(NTILES) if s_exts[j] <= P]


def _chunks_for(E, first_slot):
    plan = []
    c0 = 0
    if first_slot:
        for ch in (8, 12, 16, 24, 32, 48):
            if c0 + ch <= E:
                plan.append((c0, ch))
                c0 += ch
    while c0 < E:
        ch = min(CH, E - c0)
        plan.append((c0, ch))
        c0 += ch
    return plan


def _build(s_exts):
    f16 = mybir.dt.float16
    f32 = mybir.dt.float32
    SE = sum(s_exts)
    offs = np.cumsum([0] + list(s_exts[:-1])).tolist()
    pe_slots = _pe_slots(s_exts)
    dve_slots = [j for j in range(NTILES) if j not in pe_slots]
    SE_DVE = sum(s_exts[j] for j in dve_slots)
    PE_COLS = sum(P * s_exts[j] for j in pe_slots)
    pe_off = {}
    acc = 0
    for j in pe_slots:
        pe_off[j] = acc
        acc += P * s_exts[j]
    dve_off = {}
    acc = 0
    for j in dve_slots:
        dve_off[j] = acc
        acc += s_exts[j]

    nc = bacc.Bacc("TRN2", target_bir_lowering=False, debug=False)

    keys_d = nc.dram_tensor("keys", [P, max(SE_DVE, 1), KD], f16, kind="ExternalInput")
    ktp_d = nc.dram_tensor("ktp", [KD, max(PE_COLS, 1)], f16, kind="ExternalInput")
    qw_d = nc.dram_tensor("qw", [QD, NTILES, P + KD], f16, kind="ExternalInput")
    qt4_d = nc.dram_tensor(
        "qt4", [QD, max(len(pe_slots), 1), 4 * P], f16, kind="ExternalInput"
    )
    mp_d = nc.dram_tensor("mp", [P, SE], f32, kind="ExternalInput")
    out_d = nc.dram_tensor("out", [P, SE], f16, kind="ExternalOutput")

    with nc.allow_low_precision(reason="fp16 tree-fold; tensor_reduce tail is fp32"):
        with tile.TileContext(nc) as tc:
            with (
                tc.tile_pool(name="keys", bufs=4) as keysp,
                tc.tile_pool(name="ktpool", bufs=3) as ktpp,
                tc.tile_pool(name="prod", bufs=2) as prodp,
                tc.tile_pool(name="h16", bufs=2) as h16p,
                tc.tile_pool(name="h32", bufs=2) as h32p,
                tc.tile_pool(name="small", bufs=2) as smallp,
                tc.tile_pool(name="soft", bufs=3) as softp,
                tc.tile_pool(name="evac", bufs=3) as evacp,
                tc.tile_pool(name="qpp", bufs=NTILES) as qpp,
                tc.tile_pool(name="psum", bufs=2, space=bass.MemorySpace.PSUM) as psump,
                tc.tile_pool(name="psbank", bufs=4, space=bass.MemorySpace.PSUM) as psbankp,
            ):
                qw = smallp.tile([QD, NTILES, P + KD], f16, tag="qw")
                nc.sync.dma_start(qw[:], qw_d[:])
                mp_t = smallp.tile([P, SE], f32, tag="mp")

                # qp for ALL slots up-front (PE otherwise idle):
                # [b,k] for DVE slots, transposed [k,b] for PE slots.
                qt4 = smallp.tile([QD, max(len(pe_slots), 1), 4 * P], f16, tag="qt4")
                if pe_slots:
                    nc.scalar.dma_start(qt4[:], qt4_d[:])
                qps = {}
                for j in range(NTILES):
                    if j in pe_slots:
                        jj = pe_slots.index(j)
                        qp_ps4 = psump.tile([P, 4 * P], f32, tag="qp_ps")
                        nc.tensor.matmul(
                            qp_ps4[:], qw[:, j, P : P + KD], qt4[:, jj, :],
                            start=True, stop=True,
                        )
                        qp = qpp.tile(
                            [P, 4 * P], f16, name=f"qp{j}", tag=f"qp{j}"
                        )
                        nc.scalar.copy(qp[:], qp_ps4[:])
                    else:
                        qp_ps = psump.tile([P, KD], f32, tag="qp_ps")
                        nc.tensor.matmul(
                            qp_ps[:], qw[:, j, :P], qw[:, j, P : P + KD],
                            start=True, stop=True,
                        )
                        qp = qpp.tile([P, KD], f16, name=f"qp{j}", tag=f"qp{j}")
                        nc.scalar.copy(qp[:], qp_ps[:])
                    qps[j] = qp

                # merged chunk plan: DVE keys chunks + PE batch-block
                # chunks, interleaved 2:1 so the DVE never starves while
                # the PE stream still lands early enough to overlap.
                dve_plan = []
                for j in dve_slots:
                    for c0, ch in _chunks_for(s_exts[j], j == dve_slots[0]):
                        dve_plan.append(("dve", j, c0, ch))
                pe_plan = []
                for j in pe_slots:
                    for b0 in range(0, P, PEB):
                        pe_plan.append(("pe", j, b0, PEB))
                plan = []
                di, pi = 0, 0
                while di < len(dve_plan) or pi < len(pe_plan):
                    n_dve = 2 if di < 5 else 1
                    for _ in range(n_dve):
                        if di < len(dve_plan):
                            plan.append(dve_plan[di]); di += 1
                    if pi < len(pe_plan) and di >= 5:
                        plan.append(pe_plan[pi]); pi += 1
                    if di >= len(dve_plan) and pi < len(pe_plan):
                        plan.append(pe_plan[pi]); pi += 1

                atts = {}
                done_pos = {j: 0 for j in range(NTILES)}
                qidx = 0

                # softmax tail in two phases: phase 1 (mask + exp) at
                # slot completion, phase 2 (recip + scale + output) two
                # plan items later -- the in-order DVE queue would
                # otherwise head-block on reciprocal waiting for ACT's exp
                soft_state = {}
                pending2 = []

                def softmax_phase1(j):
                    E = s_exts[j]
                    off = offs[j]
                    att = atts[j]
                    atm = softp.tile([P, E], f32, name=f"atm{j}", tag="atm")
                    nc.vector.tensor_tensor(
                        atm[:], att[:], mp_t[:, off : off + E],
                        op=mybir.AluOpType.add,
                    )
                    e_t = softp.tile([P, E], f32, name=f"e{j}", tag="e")
                    ssum = softp.tile([P, 1], f32, name=f"ssum{j}", tag="ssum")
                    nc.scalar.activation(
                        e_t[:], atm[:], mybir.ActivationFunctionType.Exp,
                        bias=0.0, scale=1.0, accum_out=ssum[:],
                    )
                    soft_state[j] = (e_t, ssum)

                def softmax_phase2(j):
                    E = s_exts[j]
                    off = offs[j]
                    e_t, ssum = soft_state[j]
                    rec = softp.tile([P, 1], f32, name=f"rec{j}", tag="rec")
                    nc.vector.reciprocal(rec[:], ssum[:])
                    o_t = softp.tile([P, E], f16, name=f"o{j}", tag="o")
                    nc.scalar.mul(o_t[:], e_t[:], rec[:])
                    out_eng = nc.sync if j == NTILES - 1 else nc.gpsimd
                    out_eng.dma_start(out_d[:, off : off + E], o_t[:])

                def softmax_tail(j, i):
                    softmax_phase1(j)
                    pending2.append((i + 2, j))

                for i, item in enumerate(plan):
                    while pending2 and pending2[0][0] <= i:
                        softmax_phase2(pending2.pop(0)[1])
                    kind, j, a0, an = item
                    E = s_exts[j]
                    if j not in atts:
                        atts[j] = softp.tile(
                            [P, E], f32, name=f"att{j}", tag=f"att{j}"
                        )
                    att = atts[j]
                    if i == 6:
                        nc.scalar.dma_start(mp_t[:], mp_d[:])
                    dma_eng = nc.scalar if (qidx % 2 == 0) else nc.sync
                    qidx += 1

                    if kind == "dve":
                        c0, ch = a0, an
                        off = dve_off[j]
                        qp = qps[j]
                        kt = keysp.tile([P, CH, KD], f16, tag="kt")
                        dma_eng.dma_start(
                            kt[:, :ch, :], keys_d[:, off + c0 : off + c0 + ch, :]
                        )
                        prod = prodp.tile([P, CH, KD], f16, tag="prod")
                        nc.vector.tensor_tensor(
                            prod[:, :ch, :],
                            kt[:, :ch, :],
                            qp[:].unsqueeze(1).broadcast_to([P, ch, KD]),
                            op=mybir.AluOpType.mult,
                        )
                        h1 = h16p.tile([P, CH, 64], f16, tag="h1")
                        nc.vector.tensor_tensor(
                            h1[:, :ch, :], prod[:, :ch, 0:64], prod[:, :ch, 64:128],
                            op=mybir.AluOpType.add,
                        )
                        h2 = h32p.tile([P, CH, 32], f16, tag="h2")
                        nc.vector.tensor_tensor(
                            h2[:, :ch, :], h1[:, :ch, 0:32], h1[:, :ch, 32:64],
                            op=mybir.AluOpType.add,
                        )
                        h3 = h32p.tile([P, CH, 16], f16, tag="h3")
                        nc.vector.tensor_tensor(
                            h3[:, :ch, :], h2[:, :ch, 0:16], h2[:, :ch, 16:32],
                            op=mybir.AluOpType.add,
                        )
                        h4 = h16p.tile([P, CH, 8], f16, tag="h4")
                        nc.vector.tensor_tensor(
                            h4[:, :ch, :], h3[:, :ch, 0:8], h3[:, :ch, 8:16],
                            op=mybir.AluOpType.add,
                        )
                        nc.vector.tensor_reduce(
                            att[:, c0 : c0 + ch], h4[:, :ch, :],
                            axis=mybir.AxisListType.X, op=mybir.AluOpType.add,
                        )
                        done_pos[j] += ch
                        if done_pos[j] == E:
                            softmax_tail(j, i)
                    else:
                        b0 = a0
                        qpT = qps[j]
                        ktp = ktpp.tile([KD, PEB * E], f16, tag="ktp")
                        dma_eng.dma_start(
                            ktp[:],
                            ktp_d[:, pe_off[j] + b0 * E : pe_off[j] + (b0 + PEB) * E],
                        )
                        # E>128 slots split each batch into parts (0,128) +
                        # (128,E); each 16-batch group fills one PSUM bank
                        # per part: batch b -> strip r=(b%16)//4 (psum
                        # partitions 32r..32r+4, 4x-replicated stationary),
                        # block i=b%4 (free cols [128i, 128i+pw)).
                        parts = [(0, min(E, P))]
                        if E > P:
                            parts.append((P, E - P))
                        for kk in range(PEB // 16):
                            for p0, pw in parts:
                                bank = psbankp.tile([P, 4, P], f32, tag="bank")
                                for bl in range(16):
                                    b = b0 + kk * 16 + bl
                                    r, ii = bl // 4, bl % 4
                                    u = kk * 16 + bl
                                    nc.tensor.matmul(
                                        bank[32 * r : 32 * r + 4, ii, 0:pw],
                                        qpT[:, 4 * b : 4 * b + 4],
                                        ktp[:, u * E + p0 : u * E + p0 + pw],
                                        start=True, stop=True,
                                        tile_position=(0, 32 * r),
                                    )
                                # full-partition evac: compute engines cannot
                                # stride the partition dim (only rows 32r
                                # carry data; the rest is ignored)
                                ev = evacp.tile([P, 4, P], f32, tag="ev")
                                nc.scalar.copy(
                                    ev[:, :, 0:pw], bank[:, :, 0:pw]
                                )
                                # partition-crossing re-gather: dest
                                # partition 16k+4r+i <- (strip 32r, block i);
                                # DMA is the only unit that crosses partitions
                                nc.sync.dma_start(
                                    att[b0 + kk * 16 : b0 + kk * 16 + 16, p0 : p0 + pw],
                                    ev[0:97:32, 0:4, 0:pw],
                                )
                        done_pos[j] += PEB
                        if done_pos[j] == P:
                            softmax_tail(j, i)
                while pending2:
                    softmax_phase2(pending2.pop(0)[1])
    nc.compile()
    return nc


def _prep(query, keys, seq_len, w):
    query = np.asarray(query)
    keys = np.asarray(keys)
    w = np.asarray(w)
    lens = np.asarray(seq_len).reshape(B).astype(np.int64)

    order = np.argsort(-lens, kind="stable")
    gp = NCORES * P  # batches per slot across all cores
    slot_max = [int(lens[order[j * gp : (j + 1) * gp]].max()) for j in range(NTILES)]
    s_exts = tuple(min(S, max(1, m)) for m in slot_max)
    SE = sum(s_exts)
    pe_slots = _pe_slots(s_exts)
    dve_slots = [j for j in range(NTILES) if j not in pe_slots]
    SE_DVE = sum(s_exts[j] for j in dve_slots)
    PE_COLS = sum(P * s_exts[j] for j in pe_slots)

    perms = []
    for c in range(NCORES):
        perms.append(
            np.concatenate(
                [order[j * gp : (j + 1) * gp][c::NCORES] for j in range(NTILES)]
            )
        )

    keys16 = keys.astype(np.float16)
    q16 = query[:, 0, :].astype(np.float16)
    wT16 = np.ascontiguousarray(w.T).astype(np.float16)
    arange_s = np.arange(S)[None, :]

    in_maps = []
    for c in range(NCORES):
        pc = perms[c]
        ka = np.zeros((P, max(SE_DVE, 1), KD), dtype=np.float16)
        ktp = np.zeros((KD, max(PE_COLS, 1)), dtype=np.float16)
        qt4 = np.zeros((QD, max(len(pe_slots), 1), 4 * P), dtype=np.float16)
        mp = np.zeros((P, SE), dtype=np.float32)
        qw = np.empty((QD, NTILES, P + KD), dtype=np.float16)
        off_all = 0
        off_dve = 0
        off_pe = 0
        for j in range(NTILES):
            E = s_exts[j]
            rows = pc[j * P : (j + 1) * P]
            sl = np.minimum(lens[rows], E)
            blk = keys16[rows, :E, :]
            blk = np.where((arange_s[:, :E, None] < sl[:, None, None]), blk, 0)
            if j in pe_slots:
                # [k, b, s] batch-major column blocks
                ktp[:, off_pe : off_pe + P * E] = blk.transpose(2, 0, 1).reshape(
                    KD, P * E
                )
                off_pe += P * E
                qt4[:, pe_slots.index(j), :] = np.repeat(q16[rows].T, 4, axis=1)
            else:
                ka[:, off_dve : off_dve + E, :] = blk
                off_dve += E
            mp[:, off_all : off_all + E] = np.where(
                arange_s[:, :E] < sl[:, None], 0.0, np.float32(MASK_NEG)
            )
            qw[:, j, :P] = q16[rows].T
            qw[:, j, P:] = wT16
            off_all += E
        in_maps.append({"keys": ka, "ktp": ktp, "qw": qw, "qt4": qt4, "mp": mp})
    return lens, s_exts, perms, in_maps


def kernel(query, keys, seq_len, w):
    global LAST_RESULTS
    lens, s_exts, perms, in_maps = _prep(query, keys, seq_len, w)

    nc = _nc_cache.get(s_exts)
    if nc is None:
        nc = _build(s_exts)
        _nc_cache[s_exts] = nc

    res = run_bass_kernel_spmd(nc, in_maps, core_ids=list(range(NCORES)))
    LAST_RESULTS = res

    out = np.zeros((B, S), dtype=np.float32)
    for c in range(NCORES):
        dev = np.asarray(res.results[c]["out"]).astype(np.float32)
        pc = perms[c]
        off = 0
        for j in range(NTILES):
            E = s_exts[j]
            rows = pc[j * P : (j + 1) * P]
            out[rows, :E] = dev[:, off : off + E]
            off += E
    # zero masked/padded positions, then fix seq_len==0 rows (uniform).
    out = np.where(np.arange(S)[None, :] < lens[:, None], out, 0.0)
    out[lens == 0, :] = np.float32(1.0 / S)
    return out


# revision 35
# speedup vs baseline: 1.1741x; 1.1449x over previous
"""Trainium2 Bass kernel for masked attention softmax (ragged sequences).

Reference computation (per batch b):
    qp[k]   = sum_q query[b,0,q] * w[k,q]
    att[s]  = sum_k qp[k] * keys[b,s,k]
    score   = where(s < seq_len[b], att, NEG_INF)
    out[b]  = softmax(score)            # over s axis

Strategy (v3 -- DVE tree-fold for long slots + PE per-batch matvec for
short slots):
  - Data-parallel over batch across 8 cores (512 batches/core, 4 slot
    tiles of 128 on the partition dim).  Batches sorted by seq_len
    descending and dealt round-robin so slot j has the same extent E_j on
    every core; keys zero-padded to E_j.  All big tensors fp16.
  - Slots 0..1 (long, E>128) on the DVE: scalar_tensor_tensor (the v0
    kernel's op) has NO DVE perf mode, but tensor_tensor runs 2x_1P with
    all-fp16 operands, so per 64-position chunk:
      prod = kt * qp_bcast        (TT mult fp16 2x, 0-stride qp broadcast)
      fold 128 -> 64 -> 32 -> 16 -> 8   (TT add fp16 2x)
      att  = tensor_reduce(axis=X, fp32)  (fp32 accumulator finishes)
  - Slots 2..3 (E<=128) on the otherwise-idle PE: per batch one matvec
    (stationary = that batch's qpT column, moving = its keysT block) into
    PSUM partition 32r via column tile_position, 16 batches per PSUM
    bank.  ACT evacuates the 4 used rows, then an SBUF->SBUF HWDGE DMA
    re-gathers them into batch-on-partitions layout [128,E] (the one
    layout every later stage needs; DMA is the only unit that can cross
    partitions).  ~60ns/batch on PE replaces ~140ns/position on DVE.
  - Mask: att += maskpen (0 valid / -1e9 padded) before exp, so padded
    positions contribute exp(-1e9)=0 to the softmax sum exactly (a
    pad-count subtraction cancels catastrophically for tiny exp-sums).
  - softmax per slot: ACT exp with accum_out, DVE reciprocal, ACT scale,
    fp16 output.  No max-subtraction: |att| <= ~60, exp finite in fp32,
    softmax shift-invariant.  seq_len==0 rows fixed on host.
  - qp on device (PE): per slot one fp16 matmul [qT | wT] -> PSUM fp32,
    ACT cast to fp16 ([b,k] for DVE slots, [k,b] for PE slots).

  Measured on trn2 (8 cores): 75.2us best HW exec (device-level
  run-to-run variance up to ~89us on identical NEFFs under external
  contention), max rel err 1.02e-2
  (gate 2e-2; the error is dominated by the fp16 tree-fold rounding on
  the DVE slots -- the PE-slot dot products accumulate in fp32).
  Baseline STT kernel: 129.7us.  The first-slot DMA ramp grows ~1.5x per
  chunk (8,12,16,24,32,48) -- matching HBM delivery (~91ns/pos) against
  DVE consumption (~138ns/pos); a 2x ramp starves the DVE ~4us.
  Rejected (measured): fold-to-4 extra level (instr+sem overhead eats the
  384-cycle saving), deeper keys prefetch bufs=5 (SBUF contention slows
  ALL concurrent DVE ops ~1.2x), PE path for the E~150 slot via 2-part
  split (true per-matvec cost ~250ns MM + ~100ns LDWEIGHTS, no LDW
  pull-ahead with full-row stationaries -> 772 matvecs PE-bound, 102us),
  PEB=16 finer PE chunks (doubles DMA-issue instrs on the ACT ring,
  head-of-line sems 12->30us, 94us), CH=96 chunks with bufs=3 (88us),
  ACT-side reciprocal via exp(-ln(x)) (inf outputs, and slower).
  The 1:1 DVE:PE chunk interleave after the ramp + deferring the mp load
  past the ramp pulls the PE slots' softmax tails into the mid-stream
  DVE queue instead of serializing them at the end (-2.5us vs 2:1).
  Softmax tails are two-phase: mask+exp at slot completion, recip+scale+
  out two plan items later, so the in-order DVE queue never head-blocks
  on ACT's exp (stall 3.9->1.2us).  The residual ~7-8us DVE gap at
  t=7-14us is invariant under every ramp/ordering/pre-issue variant
  tried; startup traces show NEFF/runtime paging (TENSOR_LOAD,
  ACT_TABLE_LOAD) sharing the SDMA engines then -- treat it as a fixed
  runtime cost, not kernel-schedulable.
"""

import sys

import numpy as np

sys.path.insert(0, "/opt/trn_rl_repo")

import concourse.bass as bass
import concourse.tile as tile
from concourse import bacc, mybir
from concourse.bass_utils import run_bass_kernel_spmd


def _install_trace_shims():
    """The agent image lacks ``antenv.axon_hooks``, so trace=True silently
    degrades.  Recreate the module and register the ctypes NTFF hook from
    trn_agent_boot; also make artifact upload failure non-fatal."""
    try:
        import types

        import antenv
        from concourse import bass_utils as _bu

        if "antenv.axon_hooks" not in sys.modules:
            mod = types.ModuleType("antenv.axon_hooks")
            mod._hook = None
            mod.set_axon_ntff_profile_hook = lambda h: setattr(mod, "_hook", h)
            mod.get_axon_ntff_profile_hook = lambda: mod._hook
            sys.modules["antenv.axon_hooks"] = mod
            antenv.axon_hooks = mod
            from trn_agent_boot.trn_boot import _ntff_profile_via_ctypes

            mod.set_axon_ntff_profile_hook(
                _ntff_profile_via_ctypes("/opt/axon/libaxon_pjrt.so")
            )

        _orig_upload = _bu.upload_artifacts

        def _safe_upload(tmpdir):
            try:
                return _orig_upload(tmpdir)
            except Exception:
                return "local://" + str(tmpdir)

        _bu.upload_artifacts = _safe_upload
    except Exception:
        pass


_install_trace_shims()

B, S, KD, QD = 4096, 200, 128, 128
NCORES = 8
P = 128
PB = B // NCORES           # batches per core
NTILES = PB // P           # slot tiles per core
CH = 64                    # s-positions per keys DMA / DVE chunk
PEB = 32                   # batches per PE-slot DMA chunk (= 2 PSUM banks)
MASK_NEG = -1.0e9

LAST_RESULTS = None
_nc_cache = {}


def _pe_slots(s_exts):
    return [j for j in range(NTILES) if s_exts[j] <= P]


def _chunks_for(E, first_slot):
    plan = []
    c0 = 0
    if first_slot:
        for ch in (8, 12, 16, 24, 32, 48):
            if c0 + ch <= E:
                plan.append((c0, ch))
                c0 += ch
    while c0 < E:
        ch = min(CH, E - c0)
        plan.append((c0, ch))
        c0 += ch
    return plan


def _build(s_exts):
    f16 = mybir.dt.float16
    f32 = mybir.dt.float32
    SE = sum(s_exts)
    offs = np.cumsum([0] + list(s_exts[:-1])).tolist()
    pe_slots = _pe_slots(s_exts)
    dve_slots = [j for j in range(NTILES) if j not in pe_slots]
    SE_DVE = sum(s_exts[j] for j in dve_slots)
    PE_COLS = sum(P * s_exts[j] for j in pe_slots)
    pe_off = {}
    acc = 0
    for j in pe_slots:
        pe_off[j] = acc
        acc += P * s_exts[j]
    dve_off = {}
    acc = 0
    for j in dve_slots:
        dve_off[j] = acc
        acc += s_exts[j]

    nc = bacc.Bacc("TRN2", target_bir_lowering=False, debug=False)

    keys_d = nc.dram_tensor("keys", [P, max(SE_DVE, 1), KD], f16, kind="ExternalInput")
    ktp_d = nc.dram_tensor("ktp", [KD, max(PE_COLS, 1)], f16, kind="ExternalInput")
    qw_d = nc.dram_tensor("qw", [QD, NTILES, P + KD], f16, kind="ExternalInput")
    qt4_d = nc.dram_tensor(
        "qt4", [QD, max(len(pe_slots), 1), 4 * P], f16, kind="ExternalInput"
    )
    mp_d = nc.dram_tensor("mp", [P, SE], f32, kind="ExternalInput")
    out_d = nc.dram_tensor("out", [P, SE], f16, kind="ExternalOutput")

    with nc.allow_low_precision(reason="fp16 tree-fold; tensor_reduce tail is fp32"):
        with tile.TileContext(nc) as tc:
            with (
                tc.tile_pool(name="keys", bufs=4) as keysp,
                tc.tile_pool(name="ktpool", bufs=3) as ktpp,
                tc.tile_pool(name="prod", bufs=2) as prodp,
                tc.tile_pool(name="h16", bufs=2) as h16p,
                tc.tile_pool(name="h32", bufs=2) as h32p,
                tc.tile_pool(name="small", bufs=2) as smallp,
                tc.tile_pool(name="soft", bufs=3) as softp,
                tc.tile_pool(name="evac", bufs=3) as evacp,
                tc.tile_pool(name="qpp", bufs=NTILES) as qpp,
                tc.tile_pool(name="psum", bufs=2, space=bass.MemorySpace.PSUM) as psump,
                tc.tile_pool(name="psbank", bufs=4, space=bass.MemorySpace.PSUM) as psbankp,
            ):
                qw = smallp.tile([QD, NTILES, P + KD], f16, tag="qw")
                nc.sync.dma_start(qw[:], qw_d[:])
                mp_t = smallp.tile([P, SE], f32, tag="mp")

                # qp for ALL slots up-front (PE otherwise idle):
                # [b,k] for DVE slots, transposed [k,b] for PE slots.
                qt4 = smallp.tile([QD, max(len(pe_slots), 1), 4 * P], f16, tag="qt4")
                if pe_slots:
                    nc.scalar.dma_start(qt4[:], qt4_d[:])
                qps = {}
                for j in range(NTILES):
                    if j in pe_slots:
                        jj = pe_slots.index(j)
                        qp_ps4 = psump.tile([P, 4 * P], f32, tag="qp_ps")
                        nc.tensor.matmul(
                            qp_ps4[:], qw[:, j, P : P + KD], qt4[:, jj, :],
                            start=True, stop=True,
                        )
                        qp = qpp.tile(
                            [P, 4 * P], f16, name=f"qp{j}", tag=f"qp{j}"
                        )
                        nc.scalar.copy(qp[:], qp_ps4[:])
                    else:
                        qp_ps = psump.tile([P, KD], f32, tag="qp_ps")
                        nc.tensor.matmul(
                            qp_ps[:], qw[:, j, :P], qw[:, j, P : P + KD],
                            start=True, stop=True,
                        )
                        qp = qpp.tile([P, KD], f16, name=f"qp{j}", tag=f"qp{j}")
                        nc.scalar.copy(qp[:], qp_ps[:])
                    qps[j] = qp

                # merged chunk plan: DVE keys chunks + PE batch-block
                # chunks, interleaved 2:1 so the DVE never starves while
                # the PE stream still lands early enough to overlap.
                dve_plan = []
                for j in dve_slots:
                    for c0, ch in _chunks_for(s_exts[j], j == dve_slots[0]):
                        dve_plan.append(("dve", j, c0, ch))
                pe_plan = []
                for j in pe_slots:
                    for b0 in range(0, P, PEB):
                        pe_plan.append(("pe", j, b0, PEB))
                plan = []
                di, pi = 0, 0
                while di < len(dve_plan) or pi < len(pe_plan):
                    n_dve = 2 if di < 5 else 1
                    for _ in range(n_dve):
                        if di < len(dve_plan):
                            plan.append(dve_plan[di]); di += 1
                    if pi < len(pe_plan) and di >= 5:
                        plan.append(pe_plan[pi]); pi += 1
                    if di >= len(dve_plan) and pi < len(pe_plan):
                        plan.append(pe_plan[pi]); pi += 1

                atts = {}
                done_pos = {j: 0 for j in range(NTILES)}
                qidx = 0

                # softmax tail in two phases: phase 1 (mask + exp) at
                # slot completion, phase 2 (recip + scale + output) two
                # plan items later -- the in-order DVE queue would
                # otherwise head-block on reciprocal waiting for ACT's exp
                soft_state = {}
                pending2 = []

                def softmax_phase1(j):
                    E = s_exts[j]
                    off = offs[j]
                    att = atts[j]
                    atm = softp.tile([P, E], f32, name=f"atm{j}", tag="atm")
                    nc.vector.tensor_tensor(
                        atm[:], att[:], mp_t[:, off : off + E],
                        op=mybir.AluOpType.add,
                    )
                    e_t = softp.tile([P, E], f32, name=f"e{j}", tag="e")
                    ssum = softp.tile([P, 1], f32, name=f"ssum{j}", tag="ssum")
                    nc.scalar.activation(
                        e_t[:], atm[:], mybir.ActivationFunctionType.Exp,
                        bias=0.0, scale=1.0, accum_out=ssum[:],
                    )
                    soft_state[j] = (e_t, ssum)

                def softmax_phase2(j):
                    E = s_exts[j]
                    off = offs[j]
                    e_t, ssum = soft_state[j]
                    rec = softp.tile([P, 1], f32, name=f"rec{j}", tag="rec")
                    nc.vector.reciprocal(rec[:], ssum[:])
                    o_t = softp.tile([P, E], f16, name=f"o{j}", tag="o")
                    nc.scalar.mul(o_t[:], e_t[:], rec[:])
                    out_eng = nc.sync if j == NTILES - 1 else nc.gpsimd
                    out_eng.dma_start(out_d[:, off : off + E], o_t[:])

                def softmax_tail(j, i):
                    softmax_phase1(j)
                    pending2.append((i + 2, j))

                for i, item in enumerate(plan):
                    while pending2 and pending2[0][0] <= i:
                        softmax_phase2(pending2.pop(0)[1])
                    kind, j, a0, an = item
                    E = s_exts[j]
                    if j not in atts:
                        atts[j] = softp.tile(
                            [P, E], f32, name=f"att{j}", tag=f"att{j}"
                        )
                    att = atts[j]
                    if i == 6:
                        nc.scalar.dma_start(mp_t[:], mp_d[:])
                    dma_eng = nc.scalar if (qidx % 2 == 0) else nc.sync
                    qidx += 1

                    if kind == "dve":
                        c0, ch = a0, an
                        off = dve_off[j]
                        qp = qps[j]
                        kt = keysp.tile([P, CH, KD], f16, tag="kt")
                        dma_eng.dma_start(
                            kt[:, :ch, :], keys_d[:, off + c0 : off + c0 + ch, :]
                        )
                        prod = prodp.tile([P, CH, KD], f16, tag="prod")
                        nc.vector.tensor_tensor(
                            prod[:, :ch, :],
                            kt[:, :ch, :],
                            qp[:].unsqueeze(1).broadcast_to([P, ch, KD]),
                            op=mybir.AluOpType.mult,
                        )
                        h1 = h16p.tile([P, CH, 64], f16, tag="h1")
                        nc.vector.tensor_tensor(
                            h1[:, :ch, :], prod[:, :ch, 0:64], prod[:, :ch, 64:128],
                            op=mybir.AluOpType.add,
                        )
                        h2 = h32p.tile([P, CH, 32], f16, tag="h2")
                        nc.vector.tensor_tensor(
                            h2[:, :ch, :], h1[:, :ch, 0:32], h1[:, :ch, 32:64],
                            op=mybir.AluOpType.add,
                        )
                        h3 = h32p.tile([P, CH, 16], f16, tag="h3")
                        nc.vector.tensor_tensor(
                            h3[:, :ch, :], h2[:, :ch, 0:16], h2[:, :ch, 16:32],
                            op=mybir.AluOpType.add,
                        )
                        h4 = h16p.tile([P, CH, 8], f16, tag="h4")
                        nc.vector.tensor_tensor(
                            h4[:, :ch, :], h3[:, :ch, 0:8], h3[:, :ch, 8:16],
                            op=mybir.AluOpType.add,
                        )
                        nc.vector.tensor_reduce(
                            att[:, c0 : c0 + ch], h4[:, :ch, :],
                            axis=mybir.AxisListType.X, op=mybir.AluOpType.add,
                        )
                        done_pos[j] += ch
                        if done_pos[j] == E:
                            softmax_tail(j, i)
                    else:
                        b0 = a0
                        qpT = qps[j]
                        ktp = ktpp.tile([KD, PEB * E], f16, tag="ktp")
                        dma_eng.dma_start(
                            ktp[:],
                            ktp_d[:, pe_off[j] + b0 * E : pe_off[j] + (b0 + PEB) * E],
                        )
                        # E>128 slots split each batch into parts (0,128) +
                        # (128,E); each 16-batch group fills one PSUM bank
                        # per part: batch b -> strip r=(b%16)//4 (psum
                        # partitions 32r..32r+4, 4x-replicated stationary),
                        # block i=b%4 (free cols [128i, 128i+pw)).
                        parts = [(0, min(E, P))]
                        if E > P:
                            parts.append((P, E - P))
                        for kk in range(PEB // 16):
                            for p0, pw in parts:
                                bank = psbankp.tile([P, 4, P], f32, tag="bank")
                                for bl in range(16):
                                    b = b0 + kk * 16 + bl
                                    r, ii = bl // 4, bl % 4
                                    u = kk * 16 + bl
                                    nc.tensor.matmul(
                                        bank[32 * r : 32 * r + 4, ii, 0:pw],
                                        qpT[:, 4 * b : 4 * b + 4],
                                        ktp[:, u * E + p0 : u * E + p0 + pw],
                                        start=True, stop=True,
                                        tile_position=(0, 32 * r),
                                    )
                                # full-partition evac: compute engines cannot
                                # stride the partition dim (only rows 32r
                                # carry data; the rest is ignored)
                                ev = evacp.tile([P, 4, P], f32, tag="ev")
                                nc.scalar.copy(
                                    ev[:, :, 0:pw], bank[:, :, 0:pw]
                                )
                                # partition-crossing re-gather: dest
                                # partition 16k+4r+i <- (strip 32r, block i);
                                # DMA is the only unit that crosses partitions
                                nc.sync.dma_start(
                                    att[b0 + kk * 16 : b0 + kk * 16 + 16, p0 : p0 + pw],
                                    ev[0:97:32, 0:4, 0:pw],
                                )
                        done_pos[j] += PEB
                        if done_pos[j] == P:
                            softmax_tail(j, i)
                while pending2:
                    softmax_phase2(pending2.pop(0)[1])
    nc.compile()
    return nc


def _prep(query, keys, seq_len, w):
    query = np.asarray(query)
    keys = np.asarray(keys)
    w = np.asarray(w)
    lens = np.asarray(seq_len).reshape(B).astype(np.int64)

    order = np.argsort(-lens, kind="stable")
    gp = NCORES * P  # batches per slot across all cores
    slot_max = [int(lens[order[j * gp : (j + 1) * gp]].max()) for j in range(NTILES)]
    s_exts = tuple(min(S, max(1, m)) for m in slot_max)
    SE = sum(s_exts)
    pe_slots = _pe_slots(s_exts)
    dve_slots = [j for j in range(NTILES) if j not in pe_slots]
    SE_DVE = sum(s_exts[j] for j in dve_slots)
    PE_COLS = sum(P * s_exts[j] for j in pe_slots)

    perms = []
    for c in range(NCORES):
        perms.append(
            np.concatenate(
                [order[j * gp : (j + 1) * gp][c::NCORES] for j in range(NTILES)]
            )
        )

    keys16 = keys.astype(np.float16)
    q16 = query[:, 0, :].astype(np.float16)
    wT16 = np.ascontiguousarray(w.T).astype(np.float16)
    arange_s = np.arange(S)[None, :]

    in_maps = []
    for c in range(NCORES):
        pc = perms[c]
        ka = np.zeros((P, max(SE_DVE, 1), KD), dtype=np.float16)
        ktp = np.zeros((KD, max(PE_COLS, 1)), dtype=np.float16)
        qt4 = np.zeros((QD, max(len(pe_slots), 1), 4 * P), dtype=np.float16)
        mp = np.zeros((P, SE), dtype=np.float32)
        qw = np.empty((QD, NTILES, P + KD), dtype=np.float16)
        off_all = 0
        off_dve = 0
        off_pe = 0
        for j in range(NTILES):
            E = s_exts[j]
            rows = pc[j * P : (j + 1) * P]
            sl = np.minimum(lens[rows], E)
            blk = keys16[rows, :E, :]
            blk = np.where((arange_s[:, :E, None] < sl[:, None, None]), blk, 0)
            if j in pe_slots:
                # [k, b, s] batch-major column blocks
                ktp[:, off_pe : off_pe + P * E] = blk.transpose(2, 0, 1).reshape(
                    KD, P * E
                )
                off_pe += P * E
                qt4[:, pe_slots.index(j), :] = np.repeat(q16[rows].T, 4, axis=1)
            else:
                ka[:, off_dve : off_dve + E, :] = blk
                off_dve += E
            mp[:, off_all : off_all + E] = np.where(
                arange_s[:, :E] < sl[:, None], 0.0, np.float32(MASK_NEG)
            )
            qw[:, j, :P] = q16[rows].T
            qw[:, j, P:] = wT16
            off_all += E
        in_maps.append({"keys": ka, "ktp": ktp, "qw": qw, "qt4": qt4, "mp": mp})
    return lens, s_exts, perms, in_maps


def kernel(query, keys, seq_len, w):
    global LAST_RESULTS
    lens, s_exts, perms, in_maps = _prep(query, keys, seq_len, w)

    nc = _nc_cache.get(s_exts)
    if nc is None:
        nc = _build(s_exts)
        _nc_cache[s_exts] = nc

    res = run_bass_kernel_spmd(nc, in_maps, core_ids=list(range(NCORES)))
    LAST_RESULTS = res

    out = np.zeros((B, S), dtype=np.float32)
    for c in range(NCORES):
        dev = np.asarray(res.results[c]["out"]).astype(np.float32)
        pc = perms[c]
        off = 0
        for j in range(NTILES):
            E = s_exts[j]
            rows = pc[j * P : (j + 1) * P]
            out[rows, :E] = dev[:, off : off + E]
            off += E
    # zero masked/padded positions, then fix seq_len==0 rows (uniform).
    out = np.where(np.arange(S)[None, :] < lens[:, None], out, 0.0)
    out[lens == 0, :] = np.float32(1.0 / S)
    return out
